# revision 12
# baseline (speedup 1.0000x reference)
"""MoE (routed top-2 + shared expert) Trainium2 kernel, 8-core expert-parallel.

Distribution strategy (B=4,S=2048,H=1024,E=8,K=2,I=1024,NSH=2):
 - Host computes the router (gate logits / softmax / top-2 / capacity mask)
   with the same jax-on-CPU ops as the reference, then dispatches tokens:
   core c receives the tokens routed to expert c plus per-slot combine
   weights. cap is the actual max per-expert token count rounded up to 128
   (<= the reference capacity 2560), so no FLOPs are spent on padding.
 - Core c runs expert c's SwiGLU MLP on its token buffer and scales rows by
   the combine weight. The shared expert is token-parallel: core c runs the
   full shared SwiGLU on tokens [c*1024, (c+1)*1024).
 - Host scatters the weighted expert outputs back and adds the shared output.

Perf notes (all measured on HW):
 - All matmuls bf16 (l2 err ~4e-3 vs the f32 reference).
 - Every matmul is a column-split pair: two 256-wide MMs into the same PSUM
   bank sharing one stationary (LD_WEIGHTS) load. ~205ns/512rows vs ~253ns
   for plain 512-wide MMs. Consecutive MMs always hit the same PSUM bank
   (bank alternation costs ~40ns/MM). start=True zeroes the WHOLE bank, so
   only the first MM of a bank group sets it.
 - Every DMA is contiguous per partition (host pre-packs tensors into the
   exact SBUF layout). Segmented 1KB-line DMAs concurrent with matmuls
   stall the PE badly (~+40% on the whole phase).
"""

import contextlib

import numpy as np

import concourse.mybir as mybir
import concourse.tile as tile
from concourse import bacc
from concourse.bass_utils import run_bass_kernel_spmd

# Problem dims (hardcoded per spec)
B, S, H = 4, 2048, 1024
E, TOPK, I = 8, 2, 1024
NSH = 2
ISH = NSH * I            # 2048 shared intermediate
RSF = 1.0
N = B * S                # 8192 tokens
CAPM = 2560              # reference capacity = ceil(1.25 * N * TOPK / E)
CAP_T = 2176             # timing-harness cap (seed-0 max count 2078 -> x128)
TSH = N // 8             # shared-expert tokens per core
P = 128
f32 = mybir.dt.float32
bf16 = mybir.dt.bfloat16
KH = H // P              # 8 contraction subtiles over H
KI = I // P              # 8 over I
KISH = ISH // P          # 16 over ISH
FD = 512                 # PSUM bank width (fp32)
NBLK = ISH // (2 * P)    # 8 shared-gu weight blocks of 2 m-tiles
OUT_DT = bf16            # dtype of eo/so device outputs
Silu = mybir.ActivationFunctionType.Silu


def _groups(cap):
    """Token ranges of <=512 (one PSUM bank each), split in <=256 halves."""
    out = []
    o = 0
    while o < cap:
        w = min(FD, cap - o)
        halves = [(0, min(256, w))]
        if w > 256:
            halves.append((256, w - 256))
        out.append((o, w, halves))
        o += w
    return out


def _declare(nc, cap=CAP_T, kind="io"):
    """All streamed tensors are partition-major ([P, ...] with everything a
    partition needs contiguous) so DMAs are long per-partition bursts."""
    ext = dict(kind="ExternalInput") if kind == "io" else {}
    exto = dict(kind="ExternalOutput") if kind == "io" else {}
    t = {}
    t["xe_t"] = nc.dram_tensor("xe_t", [P, KH * cap], bf16, **ext)
    t["wg_t"] = nc.dram_tensor("wg_t", [P, KH * I], bf16, **ext)
    t["wu_t"] = nc.dram_tensor("wu_t", [P, KH * I], bf16, **ext)
    t["wd_t"] = nc.dram_tensor("wd_t", [P, KI * H], bf16, **ext)
    t["wv"] = nc.dram_tensor("wv", [P, cap // P], f32, **ext)
    t["xs_t"] = nc.dram_tensor("xs_t", [P, KH * TSH], bf16, **ext)
    t["wsg_t"] = nc.dram_tensor("wsg_t", [P, NBLK, KH * 2 * P], bf16, **ext)
    t["wsu_t"] = nc.dram_tensor("wsu_t", [P, NBLK, KH * 2 * P], bf16, **ext)
    t["wsd_t"] = nc.dram_tensor("wsd_t", [P, H // FD, KISH * FD], bf16, **ext)
    t["eo"] = nc.dram_tensor("eo", [cap, H], OUT_DT, **exto)
    t["so"] = nc.dram_tensor("so", [TSH, H], OUT_DT, **exto)
    return t


def _declare_internal(nc, cap=CAP_T):
    return _declare(nc, cap, kind="internal")


def _pools_routed(tc, ctx):
    return {
        "w": ctx.enter_context(tc.tile_pool(name="wR", bufs=1)),
        "x": ctx.enter_context(tc.tile_pool(name="xR", bufs=2)),
        "h": ctx.enter_context(tc.tile_pool(name="hR", bufs=2)),
        "t": ctx.enter_context(tc.tile_pool(name="tR", bufs=3)),
        "o": ctx.enter_context(tc.tile_pool(name="oR", bufs=4)),
    }


def _pools_shared(tc, ctx):
    return {
        "w": ctx.enter_context(tc.tile_pool(name="wS", bufs=1)),
        "gu": ctx.enter_context(tc.tile_pool(name="guS", bufs=2)),
        "d": ctx.enter_context(tc.tile_pool(name="dS", bufs=2)),
        "t": ctx.enter_context(tc.tile_pool(name="tS", bufs=3)),
        "o": ctx.enter_context(tc.tile_pool(name="oS", bufs=4)),
    }


def _emit_routed_weights(nc, t, pools, cap=CAP_T):
    """Load expert weights resident in SBUF (once, outside any timing loop)."""
    w = pools["w"]
    wg_sb = w.tile([P, KH, I], bf16, tag="wg")
    nc.sync.dma_start(wg_sb[:].rearrange("p k i -> p (k i)"), t["wg_t"][:])
    wu_sb = w.tile([P, KH, I], bf16, tag="wu")
    nc.sync.dma_start(wu_sb[:].rearrange("p k i -> p (k i)"), t["wu_t"][:])
    wd_sb = w.tile([P, KI, H], bf16, tag="wd")
    nc.sync.dma_start(wd_sb[:].rearrange("p k h -> p (k h)"), t["wd_t"][:])
    wv_sb = w.tile([P, cap // P], f32, tag="wv")
    nc.sync.dma_start(wv_sb[:], t["wv"][:])
    return wg_sb, wu_sb, wd_sb, wv_sb


def _emit_routed_body(nc, psum, t, pools, wsbs, cap=CAP_T, *,
                      skip_eo_dma=False, xe_res=None, dma_xe=True,
                      skip_down=False, skip_gu=False):
    wg_sb, wu_sb, wd_sb, wv_sb = wsbs
    if dma_xe:
        xe_sb = pools["x"].tile([P, KH, cap], bf16, tag="xe")
        nc.sync.dma_start(xe_sb[:].rearrange("p k t -> p (k t)"), t["xe_t"][:])
    if xe_res is not None:
        xe_sb = xe_res
    for off, w, halves in _groups(cap):
        h_sb = pools["h"].tile([P, KI, FD], bf16, tag="h")
        for m in range(KI if not skip_gu else 0):
            # col-split pairs into one PSUM bank share the stationary load;
            # start=True zeroes the WHOLE bank, so only the first MM sets it.
            ps_g = psum.tile([P, FD], f32, tag="g")
            first = True
            for k in range(KH):
                for c0, cw in halves:
                    nc.tensor.matmul(
                        ps_g[:, c0:c0 + cw], wg_sb[:, k, m * P:(m + 1) * P],
                        xe_sb[:, k, off + c0:off + c0 + cw],
                        start=first, stop=(k == KH - 1 and c0 == halves[-1][0]))
                    first = False
            ps_u = psum.tile([P, FD], f32, tag="u")
            first = True
            for k in range(KH):
                for c0, cw in halves:
                    nc.tensor.matmul(
                        ps_u[:, c0:c0 + cw], wu_sb[:, k, m * P:(m + 1) * P],
                        xe_sb[:, k, off + c0:off + c0 + cw],
                        start=first, stop=(k == KH - 1 and c0 == halves[-1][0]))
                    first = False
            sg = pools["t"].tile([P, FD], f32, tag="sg")
            nc.scalar.activation(sg[:, :w], ps_g[:, :w], Silu)
            nc.vector.tensor_mul(out=h_sb[:, m, :w], in0=sg[:, :w],
                                 in1=ps_u[:, :w])
        for tt in range(w // P if not skip_down else 0):
            j = (off + tt * P) // P
            for hn in range(H // FD):
                ps_o = psum.tile([P, FD], f32, tag="o", bufs=4)
                first = True
                for m in range(KI):
                    for c0 in (0, 256):
                        nc.tensor.matmul(
                            ps_o[:, c0:c0 + 256],
                            h_sb[:, m, tt * P:(tt + 1) * P],
                            wd_sb[:, m, hn * FD + c0:hn * FD + c0 + 256],
                            start=first, stop=(m == KI - 1 and c0 == 256))
                        first = False
                o_sb = pools["o"].tile([P, FD], OUT_DT, tag="o_sb")
                nc.vector.tensor_scalar_mul(o_sb[:], ps_o[:],
                                            wv_sb[:, j:j + 1])
                if not skip_eo_dma:
                    nc.sync.dma_start(
                        t["eo"][off + tt * P:off + (tt + 1) * P,
                                hn * FD:(hn + 1) * FD],
                        o_sb[:])


def _emit_shared_body(nc, psum, t, pools):
    xs_sb = pools["w"].tile([P, KH, TSH], bf16, tag="xs")
    nc.sync.dma_start(xs_sb[:].rearrange("p k t -> p (k t)"), t["xs_t"][:])
    hs_sb = pools["w"].tile([P, KISH, TSH], bf16, tag="hs")

    for b in range(NBLK):                      # weight blocks of 2 m-tiles
        wsg_blk = pools["gu"].tile([P, KH, 2 * P], bf16, tag="wsg")
        nc.sync.dma_start(wsg_blk[:].rearrange("p k i -> p (k i)"),
                          t["wsg_t"][:][:, b])
        wsu_blk = pools["gu"].tile([P, KH, 2 * P], bf16, tag="wsu")
        nc.sync.dma_start(wsu_blk[:].rearrange("p k i -> p (k i)"),
                          t["wsu_t"][:][:, b])
        for dm in range(2):
            m = 2 * b + dm
            for c2 in range(TSH // FD):        # 2 banks of 512 tokens
                ps_g = psum.tile([P, FD], f32, tag="g")
                first = True
                for k in range(KH):
                    for c0 in (0, 256):
                        nc.tensor.matmul(
                            ps_g[:, c0:c0 + 256],
                            wsg_blk[:, k, dm * P:(dm + 1) * P],
                            xs_sb[:, k, c2 * FD + c0:c2 * FD + c0 + 256],
                            start=first, stop=(k == KH - 1 and c0 == 256))
                        first = False
                ps_u = psum.tile([P, FD], f32, tag="u")
                first = True
                for k in range(KH):
                    for c0 in (0, 256):
                        nc.tensor.matmul(
                            ps_u[:, c0:c0 + 256],
                            wsu_blk[:, k, dm * P:(dm + 1) * P],
                            xs_sb[:, k, c2 * FD + c0:c2 * FD + c0 + 256],
                            start=first, stop=(k == KH - 1 and c0 == 256))
                        first = False
                sg = pools["t"].tile([P, FD], f32, tag="sg")
                nc.scalar.activation(sg[:], ps_g[:], Silu)
                nc.vector.tensor_mul(
                    out=hs_sb[:, m, c2 * FD:(c2 + 1) * FD],
                    in0=sg[:], in1=ps_u[:])

    wsd_blks = []
    for hn in range(H // FD):                  # both bf16 halves resident
        wsd_blk = pools["d"].tile([P, KISH, FD], bf16, tag="wsd")
        nc.sync.dma_start(wsd_blk[:].rearrange("p k h -> p (k h)"),
                          t["wsd_t"][:][:, hn])
        wsd_blks.append(wsd_blk)
    for tt in range(TSH // P):                 # 8 token tiles
        for hn in range(H // FD):
            ps_o = psum.tile([P, FD], f32, tag="o", bufs=4)
            first = True
            for m in range(KISH):
                for c0 in (0, 256):
                    nc.tensor.matmul(
                        ps_o[:, c0:c0 + 256],
                        hs_sb[:, m, tt * P:(tt + 1) * P],
                        wsd_blks[hn][:, m, c0:c0 + 256],
                        start=first, stop=(m == KISH - 1 and c0 == 256))
                    first = False
            o_sb = pools["o"].tile([P, FD], OUT_DT, tag="o_sb")
            nc.vector.tensor_copy(o_sb[:], ps_o[:])
            nc.sync.dma_start(
                t["so"][tt * P:(tt + 1) * P, hn * FD:(hn + 1) * FD], o_sb[:])


def _build_nc(cap=CAP_T):
    nc = bacc.Bacc()
    t = _declare(nc, cap)
    with tile.TileContext(nc) as tc:
        with tc.tile_pool(name="psum", bufs=2, space="PSUM") as psum:
            with contextlib.ExitStack() as rctx:
                pools = _pools_routed(tc, rctx)
                wsbs = _emit_routed_weights(nc, t, pools, cap)
                _emit_routed_body(nc, psum, t, pools, wsbs, cap)
            with contextlib.ExitStack() as sctx:
                pools = _pools_shared(tc, sctx)
                _emit_shared_body(nc, psum, t, pools)
    nc.compile()
    return nc


def _route(x, gate_w):
    """Router: mirrors the reference's jax ops (on CPU) for bit-exact top-k."""
    import jax
    import jax.numpy as jnp

    cpu = jax.devices("cpu")[0]
    with jax.default_device(cpu):
        logits = jnp.asarray(x).astype(jnp.float32) @ \
            jnp.asarray(gate_w).astype(jnp.float32).T
        scores = jax.nn.softmax(logits, axis=-1)
        topk_w, topk_idx = jax.lax.top_k(scores, TOPK)
        topk_w = topk_w / (topk_w.sum(-1, keepdims=True) + 1e-20) * RSF
        topk_w = np.asarray(topk_w)
        topk_idx = np.asarray(topk_idx)

    flat_e = topk_idx.reshape(-1).astype(np.int64)          # [N*K]
    onehot = (flat_e[:, None] == np.arange(E)[None, :]).astype(np.int32)
    pos = (np.cumsum(onehot, axis=0) - 1)[np.arange(flat_e.size), flat_e]
    keep = pos < CAPM
    return topk_w, topk_idx, flat_e, pos, keep


def _pm(a, K):
    """[K*P, X] row-major -> [P, K*X] partition-major (row r = k*P + p)."""
    KP, X = a.shape
    assert KP == K * P
    return np.ascontiguousarray(
        a.reshape(K, P, X).swapaxes(0, 1).reshape(P, K * X))


def _prepare(hidden_states, gate_w, we_gate, we_up, we_down,
             ws_gate, ws_up, ws_down):
    import ml_dtypes
    b16 = ml_dtypes.bfloat16

    x = np.asarray(hidden_states, np.float32).reshape(-1, H)
    topk_w, topk_idx, flat_e, pos, keep = _route(x, np.asarray(gate_w, np.float32))

    tok = np.repeat(np.arange(N), TOPK)
    e_s, p_s = flat_e[keep], pos[keep]
    n_s, w_s = tok[keep], topk_w.reshape(-1)[keep]

    # actual per-expert counts -> capacity actually needed (mult of 128)
    counts = np.bincount(e_s, minlength=E)
    cap = int(min(CAPM, max(256, -(-counts.max() // P) * P)))

    xb = x.astype(b16)
    xe_all = np.zeros((E, H, cap), b16)
    xe_all[e_s, :, p_s] = xb[n_s]
    wv_all = np.zeros((E, cap), np.float32)
    wv_all[e_s, p_s] = w_s

    we_gate = np.asarray(we_gate, np.float32).astype(b16)
    we_up = np.asarray(we_up, np.float32).astype(b16)
    we_down = np.asarray(we_down, np.float32).astype(b16)
    wsg_T = np.asarray(ws_gate, np.float32).astype(b16).T   # [H, ISH]
    wsu_T = np.asarray(ws_up, np.float32).astype(b16).T
    wsd_T = np.asarray(ws_down, np.float32).astype(b16).T   # [ISH, H]

    # shared weights, block-major partition-contiguous
    wsg_b = np.stack([_pm(wsg_T[:, b * 2 * P:(b + 1) * 2 * P], KH)
                      for b in range(NBLK)], axis=1)        # [P, NBLK, KH*256]
    wsu_b = np.stack([_pm(wsu_T[:, b * 2 * P:(b + 1) * 2 * P], KH)
                      for b in range(NBLK)], axis=1)
    wsd_b = np.stack([_pm(wsd_T[:, hn * FD:(hn + 1) * FD], KISH)
                      for hn in range(H // FD)], axis=1)    # [P, 2, KISH*512]

    in_maps = []
    for c in range(8):
        in_maps.append({
            "xe_t": _pm(xe_all[c], KH),
            "wg_t": _pm(np.ascontiguousarray(we_gate[c].T), KH),
            "wu_t": _pm(np.ascontiguousarray(we_up[c].T), KH),
            "wd_t": _pm(np.ascontiguousarray(we_down[c].T), KI),
            "wv": np.ascontiguousarray(wv_all[c].reshape(cap // P, P).T),
            "xs_t": _pm(np.ascontiguousarray(xb[c * TSH:(c + 1) * TSH].T), KH),
            "wsg_t": np.ascontiguousarray(wsg_b),
            "wsu_t": np.ascontiguousarray(wsu_b),
            "wsd_t": np.ascontiguousarray(wsd_b),
        })
    meta = (topk_idx, pos.reshape(N, TOPK), keep.reshape(N, TOPK), cap)
    return in_maps, meta


def _combine(results, meta, out_shape):
    topk_idx, pos2, keep2, cap = meta
    eo_all = np.stack([results[c]["eo"].astype(np.float32)
                       for c in range(8)])                   # [E, cap, H]
    y = np.concatenate([results[c]["so"].astype(np.float32)
                       for c in range(8)], axis=0)           # [N, H]
    for k in range(TOPK):
        pk = np.clip(pos2[:, k], 0, cap - 1)
        contrib = eo_all[topk_idx[:, k], pk]                # weighted on device
        y = y + np.where(keep2[:, k, None] & (pos2[:, k] < cap)[:, None],
                         contrib, np.float32(0.0))
    return y.reshape(out_shape).astype(np.float32)


def kernel(hidden_states, gate_w, we_gate, we_up, we_down,
           ws_gate, ws_up, ws_down):
    import time

    hidden_states = np.asarray(hidden_states, np.float32)
    in_maps, meta = _prepare(hidden_states, gate_w, we_gate, we_up, we_down,
                             ws_gate, ws_up, ws_down)
    nc = _build_nc(meta[3])
    res = None
    for attempt in range(3):
        try:
            res = run_bass_kernel_spmd(nc, in_maps, list(range(8)))
            break
        except Exception:
            # Transient device wedges (NRT_EXEC_UNIT_UNRECOVERABLE) have been
            # observed through the axon tunnel; back off and retry.
            if attempt == 2:
                raise
            time.sleep(15)
    return _combine(res.results, meta, hidden_states.shape)


# revision 13
# speedup vs baseline: 1.1072x; 1.1072x over previous
"""MoE (routed top-2 + shared expert) Trainium2 kernel, 8-core expert-parallel.

Distribution strategy (B=4,S=2048,H=1024,E=8,K=2,I=1024,NSH=2):
 - Host computes the router (gate logits / softmax / top-2 / capacity mask)
   with the same jax-on-CPU ops as the reference, then dispatches tokens:
   core c receives the tokens routed to expert c plus per-slot combine
   weights. cap is the actual max per-expert token count rounded up to 128
   (<= the reference capacity 2560), so no FLOPs are spent on padding.
 - Core c runs expert c's SwiGLU MLP on its token buffer and scales rows by
   the combine weight. The shared expert is token-parallel: core c runs the
   full shared SwiGLU on tokens [c*1024, (c+1)*1024).
 - Host scatters the weighted expert outputs back and adds the shared output.

Perf notes (all measured on HW):
 - All matmuls bf16 (l2 err ~4e-3 vs the f32 reference).
 - Every matmul is a column-split pair: two 256-wide MMs into the same PSUM
   bank sharing one stationary (LD_WEIGHTS) load. ~205ns/512rows vs ~253ns
   for plain 512-wide MMs. Consecutive MMs always hit the same PSUM bank
   (bank alternation costs ~40ns/MM). start=True zeroes the WHOLE bank, so
   only the first MM of a bank group sets it.
 - Every DMA is contiguous per partition (host pre-packs tensors into the
   exact SBUF layout). Segmented 1KB-line DMAs concurrent with matmuls
   stall the PE badly (~+40% on the whole phase).
"""

import contextlib

import numpy as np

import concourse.mybir as mybir
import concourse.tile as tile
from concourse import bacc
from concourse.bass_utils import run_bass_kernel_spmd

# Problem dims (hardcoded per spec)
B, S, H = 4, 2048, 1024
E, TOPK, I = 8, 2, 1024
NSH = 2
ISH = NSH * I            # 2048 shared intermediate
RSF = 1.0
N = B * S                # 8192 tokens
CAPM = 2560              # reference capacity = ceil(1.25 * N * TOPK / E)
CAP_T = 2176             # timing-harness cap (seed-0 max count 2078 -> x128)
TSH = N // 8             # shared-expert tokens per core
P = 128
f32 = mybir.dt.float32
bf16 = mybir.dt.bfloat16
KH = H // P              # 8 contraction subtiles over H
KI = I // P              # 8 over I
KISH = ISH // P          # 16 over ISH
FD = 512                 # PSUM bank width (fp32)
NBLK = ISH // (2 * P)    # 8 shared-gu weight blocks of 2 m-tiles
OUT_DT = bf16            # dtype of eo/so device outputs
Silu = mybir.ActivationFunctionType.Silu


def _groups(cap):
    """Token ranges of <=512 (one PSUM bank each), split in <=256 halves."""
    out = []
    o = 0
    while o < cap:
        w = min(FD, cap - o)
        halves = [(0, min(256, w))]
        if w > 256:
            halves.append((256, w - 256))
        out.append((o, w, halves))
        o += w
    return out


def _declare(nc, cap=CAP_T, kind="io"):
    """All streamed tensors are partition-major ([P, ...] with everything a
    partition needs contiguous) so DMAs are long per-partition bursts."""
    ext = dict(kind="ExternalInput") if kind == "io" else {}
    exto = dict(kind="ExternalOutput") if kind == "io" else {}
    t = {}
    t["xe_t"] = nc.dram_tensor("xe_t", [P, KH * cap], bf16, **ext)
    t["wg_t"] = nc.dram_tensor("wg_t", [P, KH * I], bf16, **ext)
    t["wu_t"] = nc.dram_tensor("wu_t", [P, KH * I], bf16, **ext)
    t["wd_t"] = nc.dram_tensor("wd_t", [P, KI * H], bf16, **ext)
    t["wv"] = nc.dram_tensor("wv", [P, cap // P], f32, **ext)
    t["xs_t"] = nc.dram_tensor("xs_t", [P, KH * TSH], bf16, **ext)
    t["wsg_t"] = nc.dram_tensor("wsg_t", [P, NBLK, KH * 2 * P], bf16, **ext)
    t["wsu_t"] = nc.dram_tensor("wsu_t", [P, NBLK, KH * 2 * P], bf16, **ext)
    t["wsd_t"] = nc.dram_tensor("wsd_t", [P, H // FD, KISH * FD], bf16, **ext)
    t["eo"] = nc.dram_tensor("eo", [cap, H], OUT_DT, **exto)
    t["so"] = nc.dram_tensor("so", [TSH, H], OUT_DT, **exto)
    return t


def _declare_internal(nc, cap=CAP_T):
    return _declare(nc, cap, kind="internal")


def _pools_routed(tc, ctx):
    return {
        "w": ctx.enter_context(tc.tile_pool(name="wR", bufs=1)),
        "x": ctx.enter_context(tc.tile_pool(name="xR", bufs=2)),
        "h": ctx.enter_context(tc.tile_pool(name="hR", bufs=2)),
        "t": ctx.enter_context(tc.tile_pool(name="tR", bufs=3)),
        "o": ctx.enter_context(tc.tile_pool(name="oR", bufs=4)),
    }


def _pools_shared(tc, ctx):
    return {
        "w": ctx.enter_context(tc.tile_pool(name="wS", bufs=1)),
        "gu": ctx.enter_context(tc.tile_pool(name="guS", bufs=2)),
        "d": ctx.enter_context(tc.tile_pool(name="dS", bufs=2)),
        "t": ctx.enter_context(tc.tile_pool(name="tS", bufs=3)),
        "o": ctx.enter_context(tc.tile_pool(name="oS", bufs=4)),
    }


def _emit_routed_weights(nc, t, pools, cap=CAP_T):
    """Load expert weights resident in SBUF (once, outside any timing loop)."""
    w = pools["w"]
    wg_sb = w.tile([P, KH, I], bf16, tag="wg")
    nc.sync.dma_start(wg_sb[:].rearrange("p k i -> p (k i)"), t["wg_t"][:])
    wu_sb = w.tile([P, KH, I], bf16, tag="wu")
    nc.sync.dma_start(wu_sb[:].rearrange("p k i -> p (k i)"), t["wu_t"][:])
    wd_sb = w.tile([P, KI, H], bf16, tag="wd")
    nc.sync.dma_start(wd_sb[:].rearrange("p k h -> p (k h)"), t["wd_t"][:])
    wv_sb = w.tile([P, cap // P], f32, tag="wv")
    nc.sync.dma_start(wv_sb[:], t["wv"][:])
    return wg_sb, wu_sb, wd_sb, wv_sb


GU_BUFS = 2              # PSUM buffers per g/u tag (o tag gets 8 - 2*GU_BUFS)


def _emit_routed_body(nc, psum, t, pools, wsbs, cap=CAP_T, *,
                      skip_eo_dma=False, xe_res=None, dma_xe=True,
                      skip_down=False, skip_gu=False):
    wg_sb, wu_sb, wd_sb, wv_sb = wsbs
    if dma_xe:
        xe_sb = pools["x"].tile([P, KH, cap], bf16, tag="xe")
        nc.sync.dma_start(xe_sb[:].rearrange("p k t -> p (k t)"), t["xe_t"][:])
    if xe_res is not None:
        xe_sb = xe_res
    for off, w, halves in _groups(cap):
        h_sb = pools["h"].tile([P, KI, FD], bf16, tag="h")
        for m in range(KI if not skip_gu else 0):
            # col-split pairs into one PSUM bank share the stationary load;
            # start=True zeroes the WHOLE bank, so only the first MM sets it.
            ps_g = psum.tile([P, FD], f32, tag="g", bufs=GU_BUFS)
            first = True
            for k in range(KH):
                for c0, cw in halves:
                    nc.tensor.matmul(
                        ps_g[:, c0:c0 + cw], wg_sb[:, k, m * P:(m + 1) * P],
                        xe_sb[:, k, off + c0:off + c0 + cw],
                        start=first, stop=(k == KH - 1 and c0 == halves[-1][0]))
                    first = False
            ps_u = psum.tile([P, FD], f32, tag="u", bufs=GU_BUFS)
            first = True
            for k in range(KH):
                for c0, cw in halves:
                    nc.tensor.matmul(
                        ps_u[:, c0:c0 + cw], wu_sb[:, k, m * P:(m + 1) * P],
                        xe_sb[:, k, off + c0:off + c0 + cw],
                        start=first, stop=(k == KH - 1 and c0 == halves[-1][0]))
                    first = False
            sg = pools["t"].tile([P, FD], f32, tag="sg")
            nc.scalar.activation(sg[:, :w], ps_g[:, :w], Silu)
            nc.vector.tensor_mul(out=h_sb[:, m, :w], in0=sg[:, :w],
                                 in1=ps_u[:, :w])
        for tt in range(w // P if not skip_down else 0):
            j = (off + tt * P) // P
            for hn in range(H // FD):
                ps_o = psum.tile([P, FD], f32, tag="o",
                                 bufs=8 - 2 * GU_BUFS)
                first = True
                for m in range(KI):
                    for c0 in (0, 256):
                        nc.tensor.matmul(
                            ps_o[:, c0:c0 + 256],
                            h_sb[:, m, tt * P:(tt + 1) * P],
                            wd_sb[:, m, hn * FD + c0:hn * FD + c0 + 256],
                            start=first, stop=(m == KI - 1 and c0 == 256))
                        first = False
                o_sb = pools["o"].tile([P, FD], OUT_DT, tag="o_sb")
                nc.vector.tensor_scalar_mul(o_sb[:], ps_o[:],
                                            wv_sb[:, j:j + 1])
                if not skip_eo_dma:
                    nc.sync.dma_start(
                        t["eo"][off + tt * P:off + (tt + 1) * P,
                                hn * FD:(hn + 1) * FD],
                        o_sb[:])


def _emit_shared_body(nc, psum, t, pools):
    xs_sb = pools["w"].tile([P, KH, TSH], bf16, tag="xs")
    nc.sync.dma_start(xs_sb[:].rearrange("p k t -> p (k t)"), t["xs_t"][:])
    hs_sb = pools["w"].tile([P, KISH, TSH], bf16, tag="hs")

    for b in range(NBLK):                      # weight blocks of 2 m-tiles
        wsg_blk = pools["gu"].tile([P, KH, 2 * P], bf16, tag="wsg")
        nc.sync.dma_start(wsg_blk[:].rearrange("p k i -> p (k i)"),
                          t["wsg_t"][:][:, b])
        wsu_blk = pools["gu"].tile([P, KH, 2 * P], bf16, tag="wsu")
        nc.sync.dma_start(wsu_blk[:].rearrange("p k i -> p (k i)"),
                          t["wsu_t"][:][:, b])
        for dm in range(2):
            m = 2 * b + dm
            for c2 in range(TSH // FD):        # 2 banks of 512 tokens
                ps_g = psum.tile([P, FD], f32, tag="g")
                first = True
                for k in range(KH):
                    for c0 in (0, 256):
                        nc.tensor.matmul(
                            ps_g[:, c0:c0 + 256],
                            wsg_blk[:, k, dm * P:(dm + 1) * P],
                            xs_sb[:, k, c2 * FD + c0:c2 * FD + c0 + 256],
                            start=first, stop=(k == KH - 1 and c0 == 256))
                        first = False
                ps_u = psum.tile([P, FD], f32, tag="u")
                first = True
                for k in range(KH):
                    for c0 in (0, 256):
                        nc.tensor.matmul(
                            ps_u[:, c0:c0 + 256],
                            wsu_blk[:, k, dm * P:(dm + 1) * P],
                            xs_sb[:, k, c2 * FD + c0:c2 * FD + c0 + 256],
                            start=first, stop=(k == KH - 1 and c0 == 256))
                        first = False
                sg = pools["t"].tile([P, FD], f32, tag="sg")
                nc.scalar.activation(sg[:], ps_g[:], Silu)
                nc.vector.tensor_mul(
                    out=hs_sb[:, m, c2 * FD:(c2 + 1) * FD],
                    in0=sg[:], in1=ps_u[:])

    wsd_blks = []
    for hn in range(H // FD):                  # both bf16 halves resident
        wsd_blk = pools["d"].tile([P, KISH, FD], bf16, tag="wsd")
        nc.sync.dma_start(wsd_blk[:].rearrange("p k h -> p (k h)"),
                          t["wsd_t"][:][:, hn])
        wsd_blks.append(wsd_blk)
    for tt in range(TSH // P):                 # 8 token tiles
        for hn in range(H // FD):
            ps_o = psum.tile([P, FD], f32, tag="o", bufs=4)
            first = True
            for m in range(KISH):
                for c0 in (0, 256):
                    nc.tensor.matmul(
                        ps_o[:, c0:c0 + 256],
                        hs_sb[:, m, tt * P:(tt + 1) * P],
                        wsd_blks[hn][:, m, c0:c0 + 256],
                        start=first, stop=(m == KISH - 1 and c0 == 256))
                    first = False
            o_sb = pools["o"].tile([P, FD], OUT_DT, tag="o_sb")
            nc.vector.tensor_copy(o_sb[:], ps_o[:])
            nc.sync.dma_start(
                t["so"][tt * P:(tt + 1) * P, hn * FD:(hn + 1) * FD], o_sb[:])


def _build_nc(cap=CAP_T):
    nc = bacc.Bacc()
    t = _declare(nc, cap)
    with tile.TileContext(nc) as tc:
        with tc.tile_pool(name="psum", bufs=2, space="PSUM") as psum:
            with contextlib.ExitStack() as rctx:
                pools = _pools_routed(tc, rctx)
                wsbs = _emit_routed_weights(nc, t, pools, cap)
                _emit_routed_body(nc, psum, t, pools, wsbs, cap)
            with contextlib.ExitStack() as sctx:
                pools = _pools_shared(tc, sctx)
                _emit_shared_body(nc, psum, t, pools)
    nc.compile()
    return nc


def _route(x, gate_w):
    """Router: mirrors the reference's jax ops (on CPU) for bit-exact top-k."""
    import jax
    import jax.numpy as jnp

    cpu = jax.devices("cpu")[0]
    with jax.default_device(cpu):
        logits = jnp.asarray(x).astype(jnp.float32) @ \
            jnp.asarray(gate_w).astype(jnp.float32).T
        scores = jax.nn.softmax(logits, axis=-1)
        topk_w, topk_idx = jax.lax.top_k(scores, TOPK)
        topk_w = topk_w / (topk_w.sum(-1, keepdims=True) + 1e-20) * RSF
        topk_w = np.asarray(topk_w)
        topk_idx = np.asarray(topk_idx)

    flat_e = topk_idx.reshape(-1).astype(np.int64)          # [N*K]
    onehot = (flat_e[:, None] == np.arange(E)[None, :]).astype(np.int32)
    pos = (np.cumsum(onehot, axis=0) - 1)[np.arange(flat_e.size), flat_e]
    keep = pos < CAPM
    return topk_w, topk_idx, flat_e, pos, keep


def _pm(a, K):
    """[K*P, X] row-major -> [P, K*X] partition-major (row r = k*P + p)."""
    KP, X = a.shape
    assert KP == K * P
    return np.ascontiguousarray(
        a.reshape(K, P, X).swapaxes(0, 1).reshape(P, K * X))


def _prepare(hidden_states, gate_w, we_gate, we_up, we_down,
             ws_gate, ws_up, ws_down):
    import ml_dtypes
    b16 = ml_dtypes.bfloat16

    x = np.asarray(hidden_states, np.float32).reshape(-1, H)
    topk_w, topk_idx, flat_e, pos, keep = _route(x, np.asarray(gate_w, np.float32))

    tok = np.repeat(np.arange(N), TOPK)
    e_s, p_s = flat_e[keep], pos[keep]
    n_s, w_s = tok[keep], topk_w.reshape(-1)[keep]

    # actual per-expert counts -> capacity actually needed (mult of 128)
    counts = np.bincount(e_s, minlength=E)
    cap = int(min(CAPM, max(256, -(-counts.max() // P) * P)))

    xb = x.astype(b16)
    xe_all = np.zeros((E, H, cap), b16)
    xe_all[e_s, :, p_s] = xb[n_s]
    wv_all = np.zeros((E, cap), np.float32)
    wv_all[e_s, p_s] = w_s

    we_gate = np.asarray(we_gate, np.float32).astype(b16)
    we_up = np.asarray(we_up, np.float32).astype(b16)
    we_down = np.asarray(we_down, np.float32).astype(b16)
    wsg_T = np.asarray(ws_gate, np.float32).astype(b16).T   # [H, ISH]
    wsu_T = np.asarray(ws_up, np.float32).astype(b16).T
    wsd_T = np.asarray(ws_down, np.float32).astype(b16).T   # [ISH, H]

    # shared weights, block-major partition-contiguous
    wsg_b = np.stack([_pm(wsg_T[:, b * 2 * P:(b + 1) * 2 * P], KH)
                      for b in range(NBLK)], axis=1)        # [P, NBLK, KH*256]
    wsu_b = np.stack([_pm(wsu_T[:, b * 2 * P:(b + 1) * 2 * P], KH)
                      for b in range(NBLK)], axis=1)
    wsd_b = np.stack([_pm(wsd_T[:, hn * FD:(hn + 1) * FD], KISH)
                      for hn in range(H // FD)], axis=1)    # [P, 2, KISH*512]

    in_maps = []
    for c in range(8):
        in_maps.append({
            "xe_t": _pm(xe_all[c], KH),
            "wg_t": _pm(np.ascontiguousarray(we_gate[c].T), KH),
            "wu_t": _pm(np.ascontiguousarray(we_up[c].T), KH),
            "wd_t": _pm(np.ascontiguousarray(we_down[c].T), KI),
            "wv": np.ascontiguousarray(wv_all[c].reshape(cap // P, P).T),
            "xs_t": _pm(np.ascontiguousarray(xb[c * TSH:(c + 1) * TSH].T), KH),
            "wsg_t": np.ascontiguousarray(wsg_b),
            "wsu_t": np.ascontiguousarray(wsu_b),
            "wsd_t": np.ascontiguousarray(wsd_b),
        })
    meta = (topk_idx, pos.reshape(N, TOPK), keep.reshape(N, TOPK), cap)
    return in_maps, meta


def _combine(results, meta, out_shape):
    topk_idx, pos2, keep2, cap = meta
    eo_all = np.stack([results[c]["eo"].astype(np.float32)
                       for c in range(8)])                   # [E, cap, H]
    y = np.concatenate([results[c]["so"].astype(np.float32)
                       for c in range(8)], axis=0)           # [N, H]
    for k in range(TOPK):
        pk = np.clip(pos2[:, k], 0, cap - 1)
        contrib = eo_all[topk_idx[:, k], pk]                # weighted on device
        y = y + np.where(keep2[:, k, None] & (pos2[:, k] < cap)[:, None],
                         contrib, np.float32(0.0))
    return y.reshape(out_shape).astype(np.float32)


def kernel(hidden_states, gate_w, we_gate, we_up, we_down,
           ws_gate, ws_up, ws_down):
    import time

    hidden_states = np.asarray(hidden_states, np.float32)
    in_maps, meta = _prepare(hidden_states, gate_w, we_gate, we_up, we_down,
                             ws_gate, ws_up, ws_down)
    nc = _build_nc(meta[3])
    res = None
    for attempt in range(3):
        try:
            res = run_bass_kernel_spmd(nc, in_maps, list(range(8)))
            break
        except Exception:
            # Transient device wedges (NRT_EXEC_UNIT_UNRECOVERABLE) have been
            # observed through the axon tunnel; back off and retry.
            if attempt == 2:
                raise
            time.sleep(15)
    return _combine(res.results, meta, hidden_states.shape)


# revision 14
# speedup vs baseline: 1.1583x; 1.0461x over previous
"""MoE (routed top-2 + shared expert) Trainium2 kernel, 8-core expert-parallel.

Distribution strategy (B=4,S=2048,H=1024,E=8,K=2,I=1024,NSH=2):
 - Host computes the router (gate logits / softmax / top-2 / capacity mask)
   with the same jax-on-CPU ops as the reference, then dispatches tokens:
   core c receives the tokens routed to expert c plus per-slot combine
   weights. cap is the actual max per-expert token count rounded up to 128
   (<= the reference capacity 2560), so no FLOPs are spent on padding.
 - Core c runs expert c's SwiGLU MLP on its token buffer and scales rows by
   the combine weight. The shared expert is token-parallel: core c runs the
   full shared SwiGLU on tokens [c*1024, (c+1)*1024).
 - Host scatters the weighted expert outputs back and adds the shared output.

Perf notes (all measured on HW):
 - All matmuls bf16 (l2 err ~4e-3 vs the f32 reference).
 - Every matmul is a column-split pair: two 256-wide MMs into the same PSUM
   bank sharing one stationary (LD_WEIGHTS) load. ~205ns/512rows vs ~253ns
   for plain 512-wide MMs. Consecutive MMs always hit the same PSUM bank
   (bank alternation costs ~40ns/MM). start=True zeroes the WHOLE bank, so
   only the first MM of a bank group sets it.
 - Every DMA is contiguous per partition (host pre-packs tensors into the
   exact SBUF layout). Segmented 1KB-line DMAs concurrent with matmuls
   stall the PE badly (~+40% on the whole phase).
"""

import contextlib

import numpy as np

import concourse.mybir as mybir
import concourse.tile as tile
from concourse import bacc
from concourse.bass_utils import run_bass_kernel_spmd

# Problem dims (hardcoded per spec)
B, S, H = 4, 2048, 1024
E, TOPK, I = 8, 2, 1024
NSH = 2
ISH = NSH * I            # 2048 shared intermediate
RSF = 1.0
N = B * S                # 8192 tokens
CAPM = 2560              # reference capacity = ceil(1.25 * N * TOPK / E)
CAP_T = 2176             # timing-harness cap (seed-0 max count 2078 -> x128)
TSH = N // 8             # shared-expert tokens per core
P = 128
f32 = mybir.dt.float32
bf16 = mybir.dt.bfloat16
KH = H // P              # 8 contraction subtiles over H
KI = I // P              # 8 over I
KISH = ISH // P          # 16 over ISH
FD = 512                 # PSUM bank width (fp32)
NBLK = ISH // (2 * P)    # 8 shared-gu weight blocks of 2 m-tiles
OUT_DT = bf16            # dtype of eo/so device outputs
Silu = mybir.ActivationFunctionType.Silu


def _groups(cap):
    """Token ranges of <=512 (one PSUM bank each), split in <=256 halves."""
    out = []
    o = 0
    while o < cap:
        w = min(FD, cap - o)
        halves = [(0, min(256, w))]
        if w > 256:
            halves.append((256, w - 256))
        out.append((o, w, halves))
        o += w
    return out


def _declare(nc, cap=CAP_T, kind="io"):
    """All streamed tensors are partition-major ([P, ...] with everything a
    partition needs contiguous) so DMAs are long per-partition bursts."""
    ext = dict(kind="ExternalInput") if kind == "io" else {}
    exto = dict(kind="ExternalOutput") if kind == "io" else {}
    t = {}
    t["xe_t"] = nc.dram_tensor("xe_t", [P, KH * cap], bf16, **ext)
    t["wg_t"] = nc.dram_tensor("wg_t", [P, KH * I], bf16, **ext)
    t["wu_t"] = nc.dram_tensor("wu_t", [P, KH * I], bf16, **ext)
    t["wd_t"] = nc.dram_tensor("wd_t", [P, KI * H], bf16, **ext)
    t["wv"] = nc.dram_tensor("wv", [P, cap // P], f32, **ext)
    t["xs_t"] = nc.dram_tensor("xs_t", [P, KH * TSH], bf16, **ext)
    t["wsg_t"] = nc.dram_tensor("wsg_t", [P, NBLK, KH * 2 * P], bf16, **ext)
    t["wsu_t"] = nc.dram_tensor("wsu_t", [P, NBLK, KH * 2 * P], bf16, **ext)
    t["wsd_t"] = nc.dram_tensor("wsd_t", [P, H // FD, KISH * FD], bf16, **ext)
    # outputs tile-blocked: each [P, FD] store is one contiguous DMA block
    t["eo"] = nc.dram_tensor("eo", [cap // P, H // FD, P, FD], OUT_DT, **exto)
    t["so"] = nc.dram_tensor("so", [TSH // P, H // FD, P, FD], OUT_DT, **exto)
    return t


def _declare_internal(nc, cap=CAP_T):
    return _declare(nc, cap, kind="internal")


def _pools_routed(tc, ctx):
    return {
        "w": ctx.enter_context(tc.tile_pool(name="wR", bufs=1)),
        "x": ctx.enter_context(tc.tile_pool(name="xR", bufs=2)),
        "h": ctx.enter_context(tc.tile_pool(name="hR", bufs=2)),
        "t": ctx.enter_context(tc.tile_pool(name="tR", bufs=3)),
        "o": ctx.enter_context(tc.tile_pool(name="oR", bufs=4)),
    }


def _pools_shared(tc, ctx):
    return {
        "w": ctx.enter_context(tc.tile_pool(name="wS", bufs=1)),
        "gu": ctx.enter_context(tc.tile_pool(name="guS", bufs=2)),
        "d": ctx.enter_context(tc.tile_pool(name="dS", bufs=2)),
        "t": ctx.enter_context(tc.tile_pool(name="tS", bufs=3)),
        "o": ctx.enter_context(tc.tile_pool(name="oS", bufs=4)),
    }


def _emit_routed_weights(nc, t, pools, cap=CAP_T):
    """Load expert weights resident in SBUF (once, outside any timing loop)."""
    w = pools["w"]
    wg_sb = w.tile([P, KH, I], bf16, tag="wg")
    nc.sync.dma_start(wg_sb[:].rearrange("p k i -> p (k i)"), t["wg_t"][:])
    wu_sb = w.tile([P, KH, I], bf16, tag="wu")
    nc.sync.dma_start(wu_sb[:].rearrange("p k i -> p (k i)"), t["wu_t"][:])
    wd_sb = w.tile([P, KI, H], bf16, tag="wd")
    nc.sync.dma_start(wd_sb[:].rearrange("p k h -> p (k h)"), t["wd_t"][:])
    wv_sb = w.tile([P, cap // P], f32, tag="wv")
    nc.sync.dma_start(wv_sb[:], t["wv"][:])
    return wg_sb, wu_sb, wd_sb, wv_sb


GU_BUFS = 2              # PSUM buffers per g/u tag (o tag gets 8 - 2*GU_BUFS)


def _emit_routed_body(nc, psum, t, pools, wsbs, cap=CAP_T, *,
                      skip_eo_dma=False, xe_res=None, dma_xe=True,
                      skip_down=False, skip_gu=False):
    wg_sb, wu_sb, wd_sb, wv_sb = wsbs
    if dma_xe:
        xe_sb = pools["x"].tile([P, KH, cap], bf16, tag="xe")
        nc.sync.dma_start(xe_sb[:].rearrange("p k t -> p (k t)"), t["xe_t"][:])
    if xe_res is not None:
        xe_sb = xe_res
    for off, w, halves in _groups(cap):
        h_sb = pools["h"].tile([P, KI, FD], bf16, tag="h")
        for m in range(KI if not skip_gu else 0):
            # col-split pairs into one PSUM bank share the stationary load;
            # start=True zeroes the WHOLE bank, so only the first MM sets it.
            ps_g = psum.tile([P, FD], f32, tag="g", bufs=GU_BUFS)
            first = True
            for k in range(KH):
                for c0, cw in halves:
                    nc.tensor.matmul(
                        ps_g[:, c0:c0 + cw], wg_sb[:, k, m * P:(m + 1) * P],
                        xe_sb[:, k, off + c0:off + c0 + cw],
                        start=first, stop=(k == KH - 1 and c0 == halves[-1][0]))
                    first = False
            ps_u = psum.tile([P, FD], f32, tag="u", bufs=GU_BUFS)
            first = True
            for k in range(KH):
                for c0, cw in halves:
                    nc.tensor.matmul(
                        ps_u[:, c0:c0 + cw], wu_sb[:, k, m * P:(m + 1) * P],
                        xe_sb[:, k, off + c0:off + c0 + cw],
                        start=first, stop=(k == KH - 1 and c0 == halves[-1][0]))
                    first = False
            sg = pools["t"].tile([P, FD], f32, tag="sg")
            nc.scalar.activation(sg[:, :w], ps_g[:, :w], Silu)
            nc.vector.tensor_mul(out=h_sb[:, m, :w], in0=sg[:, :w],
                                 in1=ps_u[:, :w])
        for tt in range(w // P if not skip_down else 0):
            j = (off + tt * P) // P
            for hn in range(H // FD):
                ps_o = psum.tile([P, FD], f32, tag="o",
                                 bufs=8 - 2 * GU_BUFS)
                first = True
                for m in range(KI):
                    for c0 in (0, 256):
                        nc.tensor.matmul(
                            ps_o[:, c0:c0 + 256],
                            h_sb[:, m, tt * P:(tt + 1) * P],
                            wd_sb[:, m, hn * FD + c0:hn * FD + c0 + 256],
                            start=first, stop=(m == KI - 1 and c0 == 256))
                        first = False
                o_sb = pools["o"].tile([P, FD], OUT_DT, tag="o_sb")
                nc.vector.tensor_scalar_mul(o_sb[:], ps_o[:],
                                            wv_sb[:, j:j + 1])
                if not skip_eo_dma:
                    nc.sync.dma_start(t["eo"][:][j, hn], o_sb[:])


def _emit_shared_body(nc, psum, t, pools):
    xs_sb = pools["w"].tile([P, KH, TSH], bf16, tag="xs")
    nc.sync.dma_start(xs_sb[:].rearrange("p k t -> p (k t)"), t["xs_t"][:])
    hs_sb = pools["w"].tile([P, KISH, TSH], bf16, tag="hs")

    for b in range(NBLK):                      # weight blocks of 2 m-tiles
        wsg_blk = pools["gu"].tile([P, KH, 2 * P], bf16, tag="wsg")
        nc.sync.dma_start(wsg_blk[:].rearrange("p k i -> p (k i)"),
                          t["wsg_t"][:][:, b])
        wsu_blk = pools["gu"].tile([P, KH, 2 * P], bf16, tag="wsu")
        nc.sync.dma_start(wsu_blk[:].rearrange("p k i -> p (k i)"),
                          t["wsu_t"][:][:, b])
        for dm in range(2):
            m = 2 * b + dm
            for c2 in range(TSH // FD):        # 2 banks of 512 tokens
                ps_g = psum.tile([P, FD], f32, tag="g")
                first = True
                for k in range(KH):
                    for c0 in (0, 256):
                        nc.tensor.matmul(
                            ps_g[:, c0:c0 + 256],
                            wsg_blk[:, k, dm * P:(dm + 1) * P],
                            xs_sb[:, k, c2 * FD + c0:c2 * FD + c0 + 256],
                            start=first, stop=(k == KH - 1 and c0 == 256))
                        first = False
                ps_u = psum.tile([P, FD], f32, tag="u")
                first = True
                for k in range(KH):
                    for c0 in (0, 256):
                        nc.tensor.matmul(
                            ps_u[:, c0:c0 + 256],
                            wsu_blk[:, k, dm * P:(dm + 1) * P],
                            xs_sb[:, k, c2 * FD + c0:c2 * FD + c0 + 256],
                            start=first, stop=(k == KH - 1 and c0 == 256))
                        first = False
                sg = pools["t"].tile([P, FD], f32, tag="sg")
                nc.scalar.activation(sg[:], ps_g[:], Silu)
                nc.vector.tensor_mul(
                    out=hs_sb[:, m, c2 * FD:(c2 + 1) * FD],
                    in0=sg[:], in1=ps_u[:])

    wsd_blks = []
    for hn in range(H // FD):                  # both bf16 halves resident
        wsd_blk = pools["d"].tile([P, KISH, FD], bf16, tag="wsd")
        nc.sync.dma_start(wsd_blk[:].rearrange("p k h -> p (k h)"),
                          t["wsd_t"][:][:, hn])
        wsd_blks.append(wsd_blk)
    for tt in range(TSH // P):                 # 8 token tiles
        for hn in range(H // FD):
            ps_o = psum.tile([P, FD], f32, tag="o", bufs=4)
            first = True
            for m in range(KISH):
                for c0 in (0, 256):
                    nc.tensor.matmul(
                        ps_o[:, c0:c0 + 256],
                        hs_sb[:, m, tt * P:(tt + 1) * P],
                        wsd_blks[hn][:, m, c0:c0 + 256],
                        start=first, stop=(m == KISH - 1 and c0 == 256))
                    first = False
            o_sb = pools["o"].tile([P, FD], OUT_DT, tag="o_sb")
            nc.vector.tensor_copy(o_sb[:], ps_o[:])
            nc.sync.dma_start(t["so"][:][tt, hn], o_sb[:])


def _build_nc(cap=CAP_T):
    nc = bacc.Bacc()
    t = _declare(nc, cap)
    with tile.TileContext(nc) as tc:
        with tc.tile_pool(name="psum", bufs=2, space="PSUM") as psum:
            with contextlib.ExitStack() as rctx:
                pools = _pools_routed(tc, rctx)
                wsbs = _emit_routed_weights(nc, t, pools, cap)
                _emit_routed_body(nc, psum, t, pools, wsbs, cap)
            with contextlib.ExitStack() as sctx:
                pools = _pools_shared(tc, sctx)
                _emit_shared_body(nc, psum, t, pools)
    nc.compile()
    return nc


def _route(x, gate_w):
    """Router: mirrors the reference's jax ops (on CPU) for bit-exact top-k."""
    import jax
    import jax.numpy as jnp

    cpu = jax.devices("cpu")[0]
    with jax.default_device(cpu):
        logits = jnp.asarray(x).astype(jnp.float32) @ \
            jnp.asarray(gate_w).astype(jnp.float32).T
        scores = jax.nn.softmax(logits, axis=-1)
        topk_w, topk_idx = jax.lax.top_k(scores, TOPK)
        topk_w = topk_w / (topk_w.sum(-1, keepdims=True) + 1e-20) * RSF
        topk_w = np.asarray(topk_w)
        topk_idx = np.asarray(topk_idx)

    flat_e = topk_idx.reshape(-1).astype(np.int64)          # [N*K]
    onehot = (flat_e[:, None] == np.arange(E)[None, :]).astype(np.int32)
    pos = (np.cumsum(onehot, axis=0) - 1)[np.arange(flat_e.size), flat_e]
    keep = pos < CAPM
    return topk_w, topk_idx, flat_e, pos, keep


def _pm(a, K):
    """[K*P, X] row-major -> [P, K*X] partition-major (row r = k*P + p)."""
    KP, X = a.shape
    assert KP == K * P
    return np.ascontiguousarray(
        a.reshape(K, P, X).swapaxes(0, 1).reshape(P, K * X))


def _prepare(hidden_states, gate_w, we_gate, we_up, we_down,
             ws_gate, ws_up, ws_down):
    import ml_dtypes
    b16 = ml_dtypes.bfloat16

    x = np.asarray(hidden_states, np.float32).reshape(-1, H)
    topk_w, topk_idx, flat_e, pos, keep = _route(x, np.asarray(gate_w, np.float32))

    tok = np.repeat(np.arange(N), TOPK)
    e_s, p_s = flat_e[keep], pos[keep]
    n_s, w_s = tok[keep], topk_w.reshape(-1)[keep]

    # actual per-expert counts -> capacity actually needed (mult of 128)
    counts = np.bincount(e_s, minlength=E)
    cap = int(min(CAPM, max(256, -(-counts.max() // P) * P)))

    xb = x.astype(b16)
    xe_all = np.zeros((E, H, cap), b16)
    xe_all[e_s, :, p_s] = xb[n_s]
    wv_all = np.zeros((E, cap), np.float32)
    wv_all[e_s, p_s] = w_s

    we_gate = np.asarray(we_gate, np.float32).astype(b16)
    we_up = np.asarray(we_up, np.float32).astype(b16)
    we_down = np.asarray(we_down, np.float32).astype(b16)
    wsg_T = np.asarray(ws_gate, np.float32).astype(b16).T   # [H, ISH]
    wsu_T = np.asarray(ws_up, np.float32).astype(b16).T
    wsd_T = np.asarray(ws_down, np.float32).astype(b16).T   # [ISH, H]

    # shared weights, block-major partition-contiguous
    wsg_b = np.stack([_pm(wsg_T[:, b * 2 * P:(b + 1) * 2 * P], KH)
                      for b in range(NBLK)], axis=1)        # [P, NBLK, KH*256]
    wsu_b = np.stack([_pm(wsu_T[:, b * 2 * P:(b + 1) * 2 * P], KH)
                      for b in range(NBLK)], axis=1)
    wsd_b = np.stack([_pm(wsd_T[:, hn * FD:(hn + 1) * FD], KISH)
                      for hn in range(H // FD)], axis=1)    # [P, 2, KISH*512]

    in_maps = []
    for c in range(8):
        in_maps.append({
            "xe_t": _pm(xe_all[c], KH),
            "wg_t": _pm(np.ascontiguousarray(we_gate[c].T), KH),
            "wu_t": _pm(np.ascontiguousarray(we_up[c].T), KH),
            "wd_t": _pm(np.ascontiguousarray(we_down[c].T), KI),
            "wv": np.ascontiguousarray(wv_all[c].reshape(cap // P, P).T),
            "xs_t": _pm(np.ascontiguousarray(xb[c * TSH:(c + 1) * TSH].T), KH),
            "wsg_t": np.ascontiguousarray(wsg_b),
            "wsu_t": np.ascontiguousarray(wsu_b),
            "wsd_t": np.ascontiguousarray(wsd_b),
        })
    meta = (topk_idx, pos.reshape(N, TOPK), keep.reshape(N, TOPK), cap)
    return in_maps, meta


def _combine(results, meta, out_shape):
    topk_idx, pos2, keep2, cap = meta
    def _unblk(a):                       # [T, H//FD, P, FD] -> [T*P, H]
        T_, nh, _, _ = a.shape
        return np.ascontiguousarray(
            a.transpose(0, 2, 1, 3).reshape(T_ * P, nh * FD)
        ).astype(np.float32)

    eo_all = np.stack([_unblk(results[c]["eo"]) for c in range(8)])
    y = np.concatenate([_unblk(results[c]["so"]) for c in range(8)], axis=0)
    for k in range(TOPK):
        pk = np.clip(pos2[:, k], 0, cap - 1)
        contrib = eo_all[topk_idx[:, k], pk]                # weighted on device
        y = y + np.where(keep2[:, k, None] & (pos2[:, k] < cap)[:, None],
                         contrib, np.float32(0.0))
    return y.reshape(out_shape).astype(np.float32)


def kernel(hidden_states, gate_w, we_gate, we_up, we_down,
           ws_gate, ws_up, ws_down):
    import time

    hidden_states = np.asarray(hidden_states, np.float32)
    in_maps, meta = _prepare(hidden_states, gate_w, we_gate, we_up, we_down,
                             ws_gate, ws_up, ws_down)
    nc = _build_nc(meta[3])
    res = None
    for attempt in range(3):
        try:
            res = run_bass_kernel_spmd(nc, in_maps, list(range(8)))
            break
        except Exception:
            # Transient device wedges (NRT_EXEC_UNIT_UNRECOVERABLE) have been
            # observed through the axon tunnel; back off and retry.
            if attempt == 2:
                raise
            time.sleep(15)
    return _combine(res.results, meta, hidden_states.shape)
